# revision 45
# baseline (speedup 1.0000x reference)
"""Dot-product attention on 8 Trainium2 NeuronCores.

Full inputs [B=4, H=16, S=1024, D=64] fp32. B*H = 64 heads are sharded
8-per-core (head parallel), processed in head PAIRS so the two
d=64-contraction score matmuls row-pack into PE quadrants concurrently.

Per head pair on-device:
  scores^T[k,q] = K d-major @ Q d-major     (fp32r, rows 0-63 / 64-127)
  E = exp(scores^T / sqrt(d_k))             (ScalarE PSUM->SBUF, bf16 out)
  outT+sums     = [V | 1]^T @ E             (bf16, contraction k, fp32 acc)
  r = exp(-ln(sums))                        (ScalarE, same table set as exp)
  out           = outT * (ones x r)         (fp32r bcast matmul + DVE mult)
Host side transposes Q/K to d-major when sharding and un-transposes the
d-major output, both in numpy.

Toolchain notes for this container (walrus 2026-05-04 + bass_rust skew):
 - walrus accepts at most ONE sync-wait per instruction. A JSON pass over
   the BIR inserts NoOps carrying extra waits right before the owning
   instruction (same engine, in-order => semantics preserved). The
   TileContext tail drain is patched the same way.
 - fp32r matmul operands must be *produced* as float32r (dram tensor
   dtype or instruction output dtype), not bitcast from float32.
"""

import json
from contextlib import ExitStack

import numpy as np

import concourse.bass as bass
import concourse.bass2jax as bass2jax
import concourse.mybir as mybir
import concourse.tile as tile
from concourse import bass_utils
from concourse.tile_rust import add_dep_helper
from concourse.vector_clock import ScopedClock

F32 = mybir.dt.float32
F32R = mybir.dt.float32r
BF16 = mybir.dt.bfloat16

N_CORES = 8
HEADS_PER_CORE = 8
S = 1024
D = 64
KT = S // 128  # 8 k-tiles per head

_DRAIN_MAX_WAITS = 1


def _split_drain_and_barrier(self, tick_clock, wait_clock):
    nc = self.nc
    drain_inst = nc.sync.drain()
    wait_clock.add_sem_waits(
        drain_inst.ins, ScopedClock({None: tick_clock.global_clock})
    )
    si = drain_inst.ins.sync_info
    if si is not None and si.on_wait and len(si.on_wait) > _DRAIN_MAX_WAITS:
        waits = list(si.on_wait)
        updates = list(si.on_update or [])
        drain_inst.ins.sync_info = mybir.SyncInfo(
            on_wait=waits[:_DRAIN_MAX_WAITS], on_update=[]
        )
        rest = waits[_DRAIN_MAX_WAITS:]
        for i in range(0, len(rest), _DRAIN_MAX_WAITS):
            extra = nc.sync.drain()
            extra.ins.sync_info = mybir.SyncInfo(
                on_wait=rest[i : i + _DRAIN_MAX_WAITS],
                on_update=updates if i + _DRAIN_MAX_WAITS >= len(rest) else [],
            )
    nc.all_engine_barrier()
    assert self.sems is not None
    popped = nc._tile_sem_poison_stack.pop()
    assert popped is self._sem_poison
    nc.clear_and_free_semaphores(list(self.sems.allocated().values()))
    nc.all_engine_barrier()


def _split_waits_in_bir(bir_json: bytes) -> bytes:
    """Hoist extra sync-waits onto NoOps inserted immediately before the
    owning instruction (same engine, in-order => semantics unchanged)."""
    j = json.loads(bir_json)
    n = 0
    for f in j["functions"]:
        for b in f["blocks"]:
            out = []
            for inst in b["instructions"]:
                si = inst.get("sync_info")
                waits = (si or {}).get("on_wait") or []
                if len(waits) > 1:
                    for w in waits[:-1]:
                        out.append(
                            {
                                "debug": inst.get("debug", 0),
                                "engine": inst["engine"],
                                "ins": [],
                                "outs": [],
                                "name": f"{inst['name']}-wsplit{n}",
                                "opcode": "NoOp",
                                "sync_info": {"on_update": [], "on_wait": [w]},
                            }
                        )
                        n += 1
                    si["on_wait"] = [waits[-1]]
                out.append(inst)
            b["instructions"] = out
    return json.dumps(j).encode()


_orig_compile_bir_kernel = bass_utils.compile_bir_kernel


def _compile_bir_kernel_splitting(bir_json, tmpdir, neff_name="file.neff"):
    return _orig_compile_bir_kernel(_split_waits_in_bir(bir_json), tmpdir, neff_name)


# walrus's lower_dve pass crashes on this kernel with ldw-opt enabled
ENABLE_LDW_OPT = False
_orig_run_command = bass_utils.run_command


def _run_command_ldw(argv, **kwargs):
    if ENABLE_LDW_OPT:
        argv = [
            a.replace("--enable-ldw-opt=false", "--enable-ldw-opt=true") for a in argv
        ]
    return _orig_run_command(argv, **kwargs)


def _install_patches():
    if not getattr(tile.TileContext, "_drain_split_installed", False):
        tile.TileContext._drain_and_barrier = _split_drain_and_barrier
        tile.TileContext._drain_split_installed = True
    if bass_utils.compile_bir_kernel is not _compile_bir_kernel_splitting:
        bass_utils.compile_bir_kernel = _compile_bir_kernel_splitting
        bass2jax.compile_bir_kernel = _compile_bir_kernel_splitting
        bass_utils.run_command = _run_command_ldw


# "pool_div": broadcast raw sums, divide on the idle GpSimd engine.
# "act_lnexp": r = exp(-ln(sums)) on ScalarE, broadcast r, multiply on DVE.
RECIP_MODE = "act_lnexp"


def build_nc(scale: float) -> bass.Bass:
    _install_patches()
    nc = bass.Bass(
        trn_type="TRN2", target_bir_lowering=False, debug=False, num_devices=N_CORES
    )
    # kq[pair, 0:64, 0:1024] = Q^T head 2p ; [0:64, 1024:] = K^T head 2p
    # kq[pair, 64:128, ...]  = same for head 2p+1    (d-major, fp32r)
    kq = nc.dram_tensor(
        "kq", [HEADS_PER_CORE // 2, 128, 2 * S], F32R, kind="ExternalInput"
    ).ap()
    # vext[h, p, t, j]: V[h, 128*t + p, j] for j < 64, 1.0 at j == 64 (bf16)
    vext = nc.dram_tensor(
        "vext", [HEADS_PER_CORE, 128, KT, 65], BF16, kind="ExternalInput"
    ).ap()
    # sels[r, k, m] = 1.0 where k == 32*r: selector weights that extract and
    # broadcast row 32r of a [128, .] rhs across 64 output partitions.
    sels_d = nc.dram_tensor("sels", [4, 128, D], F32R, kind="ExternalInput").ap()
    outT = nc.dram_tensor(
        "outT", [HEADS_PER_CORE, D, S], F32, kind="ExternalOutput"
    ).ap()

    with tile.TileContext(nc) as tc, ExitStack() as ctx:
        sb = ctx.enter_context(tc.tile_pool(name="sb", bufs=2))
        singles = ctx.enter_context(tc.tile_pool(name="singles", bufs=1))
        # PSUM: per-head stages [128,1024] x3 bufs = 6 banks (real double
        # buffering of the MM1->exp handoff); out/bc 1-bank tag x2 = 2 banks.
        ps_stage = ctx.enter_context(tc.tile_pool(name="ps_stage", bufs=3, space="PSUM"))
        ps_o = ctx.enter_context(tc.tile_pool(name="ps_o", bufs=2, space="PSUM"))

        NPAIR = HEADS_PER_CORE // 2
        state = {}  # pair -> (v_a, v_b, e_s, kq_s)

        def emit_mm1_stage(pair, ki, kq_s, e_s):
            st_a = ps_stage.tile([128, S], F32, tag="stage", name=f"st_a_{pair}_{ki}")
            st_b = ps_stage.tile([128, S], F32, tag="stage", name=f"st_b_{pair}_{ki}")
            st = [st_a, st_b]
            last_mm = None
            for c in range(2):
                for half in range(2):  # interleave A/B for concurrency
                    base = 64 * half
                    last_mm = nc.tensor.matmul(
                        st[half][:, c * 512 : (c + 1) * 512],
                        kq_s[base : base + 64, S + ki * 128 : S + (ki + 1) * 128],
                        kq_s[base : base + 64, c * 512 : (c + 1) * 512],
                        start=True,
                        stop=True,
                    )
            for half in range(2):
                nc.scalar.activation(
                    out=e_s[:, ki, half * S : (half + 1) * S], in_=st[half],
                    func=mybir.ActivationFunctionType.Exp, scale=scale,
                )
            return last_mm

        def emit_mm2_group(pair, half, c, o_tiles):
            v_a, v_b, e_s = state[pair][:3]
            v_s = v_a if half == 0 else v_b
            o_ps = ps_o.tile([65, 512], F32, tag="o")
            o_tiles[(half, c)] = o_ps
            for ki in range(KT):
                nc.tensor.matmul(
                    o_ps,
                    v_s[:, ki, :],
                    e_s[:, ki, half * S + c * 512 : half * S + (c + 1) * 512],
                    start=(ki == 0),
                    stop=(ki == KT - 1),
                )
            row = 2 * half + c
            nc.vector.tensor_copy(
                sums_sp[32 * row : 32 * row + 1, :], o_ps[64:65, :]
            )

        def emit_normalize(pair, half, c, o_tiles, after=None):
            h = 2 * pair + half
            row = 2 * half + c
            o_ps = o_tiles[(half, c)]
            # stash unnormalized out in SBUF, freeing the o-slot for bc
            ou_s = sb.tile([D, 512], F32, tag="ou")
            nc.vector.tensor_copy(ou_s, o_ps[0:D, :])
            bc_ps = ps_o.tile([D, 512], F32, tag="o")
            # K=64 slice (fp32r is 1 cyc/row at K<=64, 2 at K=128); the
            # selector's one-hot row lands in the same 64-partition half
            # as recip_sp row 32*`row`, and bases {0,64} are legal.
            hr = slice(0, 64) if row < 2 else slice(64, 128)
            nc.tensor.matmul(
                bc_ps, sels_s[hr, row, :], recip_sp[hr, :], start=True, stop=True
            )
            if c == 0:
                o_s = sb.tile([D, S], F32, tag=f"o_s{half}")
                o_tiles[("os", half)] = o_s
            else:
                o_s = o_tiles[("os", half)]
            # o_s half = (bc_ps * 1.0) * ou_s : fused psum read + multiply
            nc.vector.scalar_tensor_tensor(
                out=o_s[:, c * 512 : (c + 1) * 512],
                in0=bc_ps,
                scalar=1.0,
                op0=mybir.AluOpType.mult,
                in1=ou_s,
                op1=mybir.AluOpType.mult,
            )
            if c == 1:
                nc.sync.dma_start(outT[h], o_s)

        def prefetch(p):
            kq_s = sb.tile([128, 2 * S], F32R, tag="kq")
            # split: q-half + first k-tile lands first so MM1 ki=0 can start
            nc.sync.dma_start(kq_s[:, : S + 128], kq[p][:, : S + 128])
            nc.sync.dma_start(kq_s[:, S + 128 :], kq[p][:, S + 128 :])
            v_a = sb.tile([128, KT, 65], BF16, tag="va")
            nc.gpsimd.dma_start(v_a, vext[2 * p])
            v_b = sb.tile([128, KT, 65], BF16, tag="vb")
            nc.gpsimd.dma_start(v_b, vext[2 * p + 1])
            e_s = sb.tile([128, KT, 2 * S], BF16, tag="e")
            state[p] = (v_a, v_b, e_s, kq_s)

        prefetch(0)
        # constants are needed only from phase 1 on; issue them after kq(0)
        sels_s = singles.tile([128, 4, D], F32R, tag="sels")
        nc.sync.dma_start(sels_s, sels_d.rearrange("r k m -> k r m"))
        # persistent sums/recip scratch; rows {0,32,64,96} hold live data,
        # the rest stay at 1.0 so the reciprocal never produces non-finites.
        sums_sp = singles.tile([128, 512], F32, tag="sums_sp")
        nc.vector.memset(sums_sp, 1.0)
        recip_f = singles.tile([128, 512], F32, tag="recip_f")
        recip_sp = singles.tile([128, 512], F32R, tag="recip_sp")

        def emit_recip(hr):
            # custom-DVE approx reciprocal hits an ISA version skew in this
            # container's walrus; native DVE reciprocal it is; per-wave halves
            # so 2-slot MM2 accumulators never deadlock against normalize.
            with nc.allow_low_precision(reason="fp32r recip for bcast matmul"):
                nc.vector.reciprocal(out=recip_sp[hr, :], in_=sums_sp[hr, :])

        # software pipeline: phase p runs MM1+exp of pair p interleaved with
        # MM2 of pair p-1 (slots 0-3) and normalize of pair p-1 (slots 5-7
        # plus one group deferred into the next phase, giving the reciprocal
        # time before the in-order PE pipe reaches the bcast matmuls).
        groups = [(hh, cc) for hh in range(2) for cc in range(2)]
        pending = []  # (pair, half, c, o_tiles) normalizes not yet emitted
        o_state = {}
        for p in range(NPAIR + 1):
            kq_s = None
            if p < NPAIR:
                kq_s = state[p][3]

            o_tiles = {}
            for ki in range(KT):
                slot_mm1 = None
                if p < NPAIR:
                    slot_mm1 = emit_mm1_stage(p, ki, kq_s, state[p][2])
                    if ki == 3 and p + 1 < NPAIR:
                        prefetch(p + 1)
                if p >= 1:
                    if ki < 2:
                        emit_mm2_group(p - 1, *groups[ki], o_tiles)
                        if ki == 1:
                            emit_recip(slice(0, 64))
                    elif ki < 4:
                        emit_normalize(p - 1, *groups[ki - 2], o_tiles)
                    elif ki < 6:
                        emit_mm2_group(p - 1, *groups[ki - 2], o_tiles)
                        if ki == 5:
                            emit_recip(slice(64, 128))
                    else:
                        emit_normalize(p - 1, *groups[ki - 4], o_tiles)
        while pending:
            emit_normalize(*pending.pop(0))

    return nc


def _shard_inputs(queries, keys, values):
    """Full [4,16,1024,64] fp32 -> per-core kq (fp32r) / vext (bf16)."""
    import ml_dtypes

    q = np.ascontiguousarray(queries, dtype=np.float32).reshape(64, S, D)
    k = np.ascontiguousarray(keys, dtype=np.float32).reshape(64, S, D)
    v = np.ascontiguousarray(values, dtype=np.float32).reshape(64, S, D)

    qT = q.transpose(0, 2, 1)  # [64, D, S]
    kT = k.transpose(0, 2, 1)

    kq = np.empty((64 // 2, 128, 2 * S), np.float32)
    kq[:, 0:64, 0:S] = qT[0::2]
    kq[:, 0:64, S:] = kT[0::2]
    kq[:, 64:128, 0:S] = qT[1::2]
    kq[:, 64:128, S:] = kT[1::2]

    vext = np.empty((64, 128, KT, 65), ml_dtypes.bfloat16)
    vext[..., 64] = 1.0
    vext[..., :64] = v.reshape(64, KT, 128, D).transpose(0, 2, 1, 3)

    sels = np.zeros((4, 128, D), np.float32)
    for r in range(4):
        sels[r, 32 * r, :] = 1.0

    in_maps = []
    for c in range(N_CORES):
        in_maps.append(
            {
                "kq": np.ascontiguousarray(kq[c * 4 : (c + 1) * 4]),
                "vext": np.ascontiguousarray(vext[c * 8 : (c + 1) * 8]),
                "sels": sels,
            }
        )
    return in_maps


_CACHE = {}


def _get_nc(scale: float) -> bass.Bass:
    if scale not in _CACHE:
        _CACHE[scale] = build_nc(scale)
    return _CACHE[scale]


def run(queries, keys, values, d_k, trace=False, trace_kwargs=None):
    scale = float(1.0 / np.sqrt(np.float32(d_k)))
    nc = _get_nc(scale)
    in_maps = _shard_inputs(queries, keys, values)
    res = bass_utils.run_bass_kernel_spmd(
        nc,
        in_maps,
        core_ids=list(range(N_CORES)),
        trace=trace,
        **(trace_kwargs or {}),
    )
    outT = np.stack([r["outT"] for r in res.results])  # [8, 8, D, S]
    out = outT.reshape(64, D, S).transpose(0, 2, 1)  # [64, S, D]
    out = np.ascontiguousarray(out).reshape(4, 16, S, D).astype(np.float32)
    return out, res


def kernel(queries, keys, values, d_k):
    out, _ = run(queries, keys, values, d_k, trace=False)
    return out


# revision 46
# speedup vs baseline: 1.2320x; 1.2320x over previous
"""Dot-product attention on 8 Trainium2 NeuronCores.

Full inputs [B=4, H=16, S=1024, D=64] fp32. B*H = 64 heads are sharded
8-per-core (head parallel), processed in head PAIRS so the two
d=64-contraction score matmuls row-pack into PE quadrants concurrently.

Per head pair on-device:
  scores^T[k,q] = K d-major @ Q d-major     (fp32r, rows 0-63 / 64-127)
  E = exp(scores^T / sqrt(d_k))             (ScalarE PSUM->SBUF, bf16 out)
  outT+sums     = [V | 1]^T @ E             (bf16, contraction k, fp32 acc)
  r = exp(-ln(sums))                        (ScalarE, same table set as exp)
  out           = outT * (ones x r)         (fp32r bcast matmul + DVE mult)
Host side transposes Q/K to d-major when sharding and un-transposes the
d-major output, both in numpy.

Toolchain notes for this container (walrus 2026-05-04 + bass_rust skew):
 - walrus accepts at most ONE sync-wait per instruction. A JSON pass over
   the BIR inserts NoOps carrying extra waits right before the owning
   instruction (same engine, in-order => semantics preserved). The
   TileContext tail drain is patched the same way.
 - fp32r matmul operands must be *produced* as float32r (dram tensor
   dtype or instruction output dtype), not bitcast from float32.
"""

import json
from contextlib import ExitStack

import numpy as np

import concourse.bass as bass
import concourse.bass2jax as bass2jax
import concourse.mybir as mybir
import concourse.tile as tile
from concourse import bass_utils
from concourse.tile_rust import add_dep_helper
from concourse.vector_clock import ScopedClock

F32 = mybir.dt.float32
F32R = mybir.dt.float32r
BF16 = mybir.dt.bfloat16

N_CORES = 8
HEADS_PER_CORE = 8
S = 1024
D = 64
KT = S // 128  # 8 k-tiles per head

_DRAIN_MAX_WAITS = 1


def _split_drain_and_barrier(self, tick_clock, wait_clock):
    nc = self.nc
    drain_inst = nc.sync.drain()
    wait_clock.add_sem_waits(
        drain_inst.ins, ScopedClock({None: tick_clock.global_clock})
    )
    si = drain_inst.ins.sync_info
    if si is not None and si.on_wait and len(si.on_wait) > _DRAIN_MAX_WAITS:
        waits = list(si.on_wait)
        updates = list(si.on_update or [])
        drain_inst.ins.sync_info = mybir.SyncInfo(
            on_wait=waits[:_DRAIN_MAX_WAITS], on_update=[]
        )
        rest = waits[_DRAIN_MAX_WAITS:]
        for i in range(0, len(rest), _DRAIN_MAX_WAITS):
            extra = nc.sync.drain()
            extra.ins.sync_info = mybir.SyncInfo(
                on_wait=rest[i : i + _DRAIN_MAX_WAITS],
                on_update=updates if i + _DRAIN_MAX_WAITS >= len(rest) else [],
            )
    nc.all_engine_barrier()
    assert self.sems is not None
    popped = nc._tile_sem_poison_stack.pop()
    assert popped is self._sem_poison
    nc.clear_and_free_semaphores(list(self.sems.allocated().values()))
    nc.all_engine_barrier()


def _split_waits_in_bir(bir_json: bytes) -> bytes:
    """Hoist extra sync-waits onto NoOps inserted immediately before the
    owning instruction (same engine, in-order => semantics unchanged)."""
    j = json.loads(bir_json)
    n = 0
    for f in j["functions"]:
        for b in f["blocks"]:
            out = []
            for inst in b["instructions"]:
                si = inst.get("sync_info")
                waits = (si or {}).get("on_wait") or []
                if len(waits) > 1:
                    for w in waits[:-1]:
                        out.append(
                            {
                                "debug": inst.get("debug", 0),
                                "engine": inst["engine"],
                                "ins": [],
                                "outs": [],
                                "name": f"{inst['name']}-wsplit{n}",
                                "opcode": "NoOp",
                                "sync_info": {"on_update": [], "on_wait": [w]},
                            }
                        )
                        n += 1
                    si["on_wait"] = [waits[-1]]
                out.append(inst)
            b["instructions"] = out
    return json.dumps(j).encode()


_orig_compile_bir_kernel = bass_utils.compile_bir_kernel


def _compile_bir_kernel_splitting(bir_json, tmpdir, neff_name="file.neff"):
    return _orig_compile_bir_kernel(_split_waits_in_bir(bir_json), tmpdir, neff_name)


# walrus's lower_dve pass crashes on this kernel with ldw-opt enabled
ENABLE_LDW_OPT = False
_orig_run_command = bass_utils.run_command


def _run_command_ldw(argv, **kwargs):
    if ENABLE_LDW_OPT:
        argv = [
            a.replace("--enable-ldw-opt=false", "--enable-ldw-opt=true") for a in argv
        ]
    return _orig_run_command(argv, **kwargs)


def _install_patches():
    if not getattr(tile.TileContext, "_drain_split_installed", False):
        tile.TileContext._drain_and_barrier = _split_drain_and_barrier
        tile.TileContext._drain_split_installed = True
    if bass_utils.compile_bir_kernel is not _compile_bir_kernel_splitting:
        bass_utils.compile_bir_kernel = _compile_bir_kernel_splitting
        bass2jax.compile_bir_kernel = _compile_bir_kernel_splitting
        bass_utils.run_command = _run_command_ldw


# "pool_div": broadcast raw sums, divide on the idle GpSimd engine.
# "act_lnexp": r = exp(-ln(sums)) on ScalarE, broadcast r, multiply on DVE.
RECIP_MODE = "act_lnexp"


def build_nc(scale: float) -> bass.Bass:
    _install_patches()
    nc = bass.Bass(
        trn_type="TRN2", target_bir_lowering=False, debug=False, num_devices=N_CORES
    )
    # kq[pair, 0:64, 0:1024] = Q^T head 2p ; [0:64, 1024:] = K^T head 2p
    # kq[pair, 64:128, ...]  = same for head 2p+1    (d-major, fp32r)
    kq = nc.dram_tensor(
        "kq", [HEADS_PER_CORE // 2, 128, 2 * S], F32R, kind="ExternalInput"
    ).ap()
    # vext[h, p, t, j]: V[h, 128*t + p, j] for j < 64, 1.0 at j == 64 (bf16)
    vext = nc.dram_tensor(
        "vext", [HEADS_PER_CORE, 128, KT, 65], BF16, kind="ExternalInput"
    ).ap()
    # sels[r, k, m] = 1.0 where k == 32*r: selector weights that extract and
    # broadcast row 32r of a [128, .] rhs across 64 output partitions.
    sels_d = nc.dram_tensor("sels", [4, 128, D], F32R, kind="ExternalInput").ap()
    outT = nc.dram_tensor(
        "outT", [HEADS_PER_CORE, D, S], F32, kind="ExternalOutput"
    ).ap()

    with tile.TileContext(nc) as tc, ExitStack() as ctx:
        sb = ctx.enter_context(tc.tile_pool(name="sb", bufs=2))
        singles = ctx.enter_context(tc.tile_pool(name="singles", bufs=1))
        # PSUM: pair stage [128,2048] = 4 banks; out/bc share a 1-bank tag
        # with 4 slots = 4 banks. Total 8.
        ps_stage = ctx.enter_context(tc.tile_pool(name="ps_stage", bufs=1, space="PSUM"))
        ps_o = ctx.enter_context(tc.tile_pool(name="ps_o", bufs=4, space="PSUM"))

        NPAIR = HEADS_PER_CORE // 2
        state = {}  # pair -> (v_a, v_b, e_s, kq_s)

        def emit_mm1_stage(pair, ki, kq_s, e_s):
            stage = ps_stage.tile([128, 2 * S], F32, tag="stage")
            last_mm = None
            for c in range(2):
                for half in range(2):  # interleave A/B for concurrency
                    base = 64 * half
                    last_mm = nc.tensor.matmul(
                        stage[:, half * S + c * 512 : half * S + (c + 1) * 512],
                        kq_s[base : base + 64, S + ki * 128 : S + (ki + 1) * 128],
                        kq_s[base : base + 64, c * 512 : (c + 1) * 512],
                        start=True,
                        stop=True,
                    )
            nc.scalar.activation(
                out=e_s[:, ki, :], in_=stage,
                func=mybir.ActivationFunctionType.Exp, scale=scale,
            )
            return last_mm

        def emit_mm2_group(pair, half, c, o_tiles):
            v_a, v_b, e_s = state[pair][:3]
            v_s = v_a if half == 0 else v_b
            o_ps = ps_o.tile([65, 512], F32, tag="o")
            o_tiles[(half, c)] = o_ps
            for ki in range(KT):
                nc.tensor.matmul(
                    o_ps,
                    v_s[:, ki, :],
                    e_s[:, ki, half * S + c * 512 : half * S + (c + 1) * 512],
                    start=(ki == 0),
                    stop=(ki == KT - 1),
                )
            row = 2 * half + c
            nc.vector.tensor_copy(
                sums_sp[32 * row : 32 * row + 1, :], o_ps[64:65, :]
            )

        def emit_normalize(pair, half, c, o_tiles, after=None):
            h = 2 * pair + half
            row = 2 * half + c
            o_ps = o_tiles[(half, c)]
            # stash unnormalized out in SBUF, freeing the o-slot for bc
            ou_s = sb.tile([D, 512], F32, tag="ou")
            nc.vector.tensor_copy(ou_s, o_ps[0:D, :])
            bc_ps = ps_o.tile([D, 512], F32, tag="o")
            # K=64 slice (fp32r is 1 cyc/row at K<=64, 2 at K=128); the
            # selector's one-hot row lands in the same 64-partition half
            # as recip_sp row 32*`row`, and bases {0,64} are legal.
            hr = slice(0, 64) if row < 2 else slice(64, 128)
            nc.tensor.matmul(
                bc_ps, sels_s[hr, row, :], recip_sp[hr, :], start=True, stop=True
            )
            if c == 0:
                o_s = sb.tile([D, S], F32, tag=f"o_s{half}")
                o_tiles[("os", half)] = o_s
            else:
                o_s = o_tiles[("os", half)]
            # o_s half = (bc_ps * 1.0) * ou_s : fused psum read + multiply
            nc.vector.scalar_tensor_tensor(
                out=o_s[:, c * 512 : (c + 1) * 512],
                in0=bc_ps,
                scalar=1.0,
                op0=mybir.AluOpType.mult,
                in1=ou_s,
                op1=mybir.AluOpType.mult,
            )
            if c == 1:
                nc.sync.dma_start(outT[h], o_s)

        def prefetch(p):
            kq_s = sb.tile([128, 2 * S], F32R, tag="kq")
            # split: q-half + first k-tile lands first so MM1 ki=0 can start
            nc.sync.dma_start(kq_s[:, : S + 128], kq[p][:, : S + 128])
            nc.sync.dma_start(kq_s[:, S + 128 :], kq[p][:, S + 128 :])
            v_a = sb.tile([128, KT, 65], BF16, tag="va")
            nc.gpsimd.dma_start(v_a, vext[2 * p])
            v_b = sb.tile([128, KT, 65], BF16, tag="vb")
            nc.gpsimd.dma_start(v_b, vext[2 * p + 1])
            e_s = sb.tile([128, KT, 2 * S], BF16, tag="e")
            state[p] = (v_a, v_b, e_s, kq_s)

        prefetch(0)
        # constants are needed only from phase 1 on; issue them after kq(0)
        sels_s = singles.tile([128, 4, D], F32R, tag="sels")
        nc.sync.dma_start(sels_s, sels_d.rearrange("r k m -> k r m"))
        # persistent sums/recip scratch; rows {0,32,64,96} hold live data,
        # the rest stay at 1.0 so the reciprocal never produces non-finites.
        sums_sp = singles.tile([128, 512], F32, tag="sums_sp")
        nc.vector.memset(sums_sp, 1.0)
        recip_f = singles.tile([128, 512], F32, tag="recip_f")
        recip_sp = singles.tile([128, 512], F32R, tag="recip_sp")

        def emit_recip():
            # custom-DVE approx reciprocal hits an ISA version skew in this
            # container's walrus; native DVE reciprocal (~3.3us) it is.
            with nc.allow_low_precision(reason="fp32r recip for bcast matmul"):
                nc.vector.reciprocal(out=recip_sp, in_=sums_sp)

        # software pipeline: phase p runs MM1+exp of pair p interleaved with
        # MM2 of pair p-1 (slots 0-3) and normalize of pair p-1 (slots 5-7
        # plus one group deferred into the next phase, giving the reciprocal
        # time before the in-order PE pipe reaches the bcast matmuls).
        groups = [(hh, cc) for hh in range(2) for cc in range(2)]
        pending = []  # (pair, half, c, o_tiles) normalizes not yet emitted
        o_state = {}
        for p in range(NPAIR + 1):
            kq_s = None
            if p < NPAIR:
                kq_s = state[p][3]

            o_tiles = {}
            for ki in range(KT):
                slot_mm1 = None
                if p < NPAIR:
                    slot_mm1 = emit_mm1_stage(p, ki, kq_s, state[p][2])
                    if ki == 3 and p + 1 < NPAIR:
                        prefetch(p + 1)
                if ki == 0 and pending:
                    emit_normalize(*pending.pop(0))
                if p >= 1:
                    if ki < 4:
                        emit_mm2_group(p - 1, *groups[ki], o_tiles)
                        if ki == 3:
                            emit_recip()
                    elif ki >= 5:
                        pending.append((p - 1, *groups[ki - 5], o_tiles))
                        emit_normalize(*pending.pop(0))
            if p >= 1:
                pending.append((p - 1, *groups[3], o_tiles))
        while pending:
            emit_normalize(*pending.pop(0))

    return nc


def _shard_inputs(queries, keys, values):
    """Full [4,16,1024,64] fp32 -> per-core kq (fp32r) / vext (bf16)."""
    import ml_dtypes

    q = np.ascontiguousarray(queries, dtype=np.float32).reshape(64, S, D)
    k = np.ascontiguousarray(keys, dtype=np.float32).reshape(64, S, D)
    v = np.ascontiguousarray(values, dtype=np.float32).reshape(64, S, D)

    qT = q.transpose(0, 2, 1)  # [64, D, S]
    kT = k.transpose(0, 2, 1)

    kq = np.empty((64 // 2, 128, 2 * S), np.float32)
    kq[:, 0:64, 0:S] = qT[0::2]
    kq[:, 0:64, S:] = kT[0::2]
    kq[:, 64:128, 0:S] = qT[1::2]
    kq[:, 64:128, S:] = kT[1::2]

    vext = np.empty((64, 128, KT, 65), ml_dtypes.bfloat16)
    vext[..., 64] = 1.0
    vext[..., :64] = v.reshape(64, KT, 128, D).transpose(0, 2, 1, 3)

    sels = np.zeros((4, 128, D), np.float32)
    for r in range(4):
        sels[r, 32 * r, :] = 1.0

    in_maps = []
    for c in range(N_CORES):
        in_maps.append(
            {
                "kq": np.ascontiguousarray(kq[c * 4 : (c + 1) * 4]),
                "vext": np.ascontiguousarray(vext[c * 8 : (c + 1) * 8]),
                "sels": sels,
            }
        )
    return in_maps


_CACHE = {}


def _get_nc(scale: float) -> bass.Bass:
    if scale not in _CACHE:
        _CACHE[scale] = build_nc(scale)
    return _CACHE[scale]


def run(queries, keys, values, d_k, trace=False, trace_kwargs=None):
    scale = float(1.0 / np.sqrt(np.float32(d_k)))
    nc = _get_nc(scale)
    in_maps = _shard_inputs(queries, keys, values)
    res = bass_utils.run_bass_kernel_spmd(
        nc,
        in_maps,
        core_ids=list(range(N_CORES)),
        trace=trace,
        **(trace_kwargs or {}),
    )
    outT = np.stack([r["outT"] for r in res.results])  # [8, 8, D, S]
    out = outT.reshape(64, D, S).transpose(0, 2, 1)  # [64, S, D]
    out = np.ascontiguousarray(out).reshape(4, 16, S, D).astype(np.float32)
    return out, res


def kernel(queries, keys, values, d_k):
    out, _ = run(queries, keys, values, d_k, trace=False)
    return out
